# revision 1
# baseline (speedup 1.0000x reference)
"""AnalyticGaussianVelocity Trainium2 kernel, 8 NeuronCores.

Math (reference):
    a=t, b=1-t
    logit_n = -(1/(2b^2)) * (|x|^2 - 2a x.y_n + a^2 |y_n|^2)
    v = -(1/b) x + (1 + a/b) * softmax(logit) @ dataset

Device-side per core (dataset sharded along N, padded 50000->50176, 6272/core,
free-dim chunks 11x512+384+256):
    G_n   = x.y_n + u*(|y_n|^2 - 512)   u=-a/2    (4 f32r matmuls -> PSUM;
                                                   DVE scalar_tensor_tensor
                                                   adds the rank-1 term from a
                                                   row-broadcast beta tile and
                                                   drains PSUM->SBUF)
    logit'_n = c1 * G_n, c1 = a/b^2                (folded into the exp's
                                                    per-partition scale)
    m     = max_n logit'_n                         (Pool-engine chunk maxes)
    P_n   = exp(logit'_n - m), l = sum_n P_n       (ACT exp with accum)
    S     = P @ ds_shard                           (DMA-XBAR-transposed P tiles,
                                                    bf16 matmuls)
Cross-core combine (flash-attention style), on device when COMBINE=True:
    M = AllReduce-max(m);  w = exp(m - M)
    [Sg | lg] = ReduceScatter-add of [w*q2*S | w*l]   (each core gets its B/8 rows)
    v_rows = q1*x_rows + Sg / lg                      (q1=-1/b, q2=1+a/b)
    host concatenates the 8 row-shards.

Padding rows are the constant 2048.0 -> giant |y|^2 -> logit ~ -1e7 -> weight 0.
"""

import numpy as np
import ml_dtypes

import concourse.bass as bass
from concourse import bacc
import concourse.mybir as mybir
import concourse.tile as tile
from concourse.bass_utils import run_bass_kernel_spmd

F32 = mybir.dt.float32
F32R = mybir.dt.float32r
BF16 = mybir.dt.bfloat16
BF = ml_dtypes.bfloat16

B, D, N = 1024, 512, 50000
NCORES = 8
NPAD = 50176                      # 8 * 6272, multiple of 128
NSH = NPAD // NCORES              # 6272 per core
KD = D // 128                     # 4 contraction tiles for logits matmul
BT = B // 128                     # 8 batch tiles
CHUNKS = [512] * 11 + [384, 256]  # free-dim chunks of NSH (all >=256: full-rate f32r)
NK2 = NSH // 128                  # 49 contraction tiles for P @ ds
PADVAL = 2048.0
X = mybir.AxisListType.X

# True: |y|^2 rank-1 term via split-bf16 matmul on PE (5th mm1 matmul, more
# SBUF headroom). False: via DVE scalar_tensor_tensor from a row-broadcast
# f32 beta tile (frees ~21us of PE, costs 24.5KB SBUF -> smaller G pool).
RANK1_ON_PE = True
TSPLIT = 1          # tiles < TSPLIT transpose P on the PE, rest via DMA XBAR
PRE = 1 if not RANK1_ON_PE else 3   # tile-1 mm1 chunks pulled into startup
GBUFS = len(CHUNKS) + PRE + (0 if not RANK1_ON_PE else 0)


def _build(combine=True):
    nc = bacc.Bacc("TRN2", target_bir_lowering=False, debug=False,
                   num_devices=NCORES, dynamic_dma_scratch_size=512)

    xT = nc.declare_dram_parameter("xT", [KD, 128, B], F32R, isOutput=False)
    dsT = nc.declare_dram_parameter("dsT", [KD, 128, NSH], F32R, isOutput=False)
    dsn = nc.declare_dram_parameter("ds_nat", [NK2, 128, D], BF16, isOutput=False)
    c1d = nc.declare_dram_parameter("c1", [128, BT], F32, isOutput=False)
    idd = nc.declare_dram_parameter("ident", [128, 128], BF16, isOutput=False)
    if RANK1_ON_PE:
        r1l = nc.declare_dram_parameter("r1_lhsT", [3, B], BF16, isOutput=False)
        r1r = nc.declare_dram_parameter("r1_rhs", [3, NSH], BF16, isOutput=False)
    else:
        beta = nc.declare_dram_parameter("beta_bc", [128, NSH], F32,
                                         isOutput=False)
        ud = nc.declare_dram_parameter("u", [128, BT], F32, isOutput=False)
    if combine:
        q2d = nc.declare_dram_parameter("q2", [128, BT], F32, isOutput=False)
        xq1 = nc.declare_dram_parameter("xq1", [128, D], F32, isOutput=False)
        vout = nc.declare_dram_parameter("out", [128, D], F32, isOutput=True)
        S_loc = nc.dram_tensor("S_loc", [B, D], F32)
        mb = nc.dram_tensor("m_bounce", [128, BT], F32)
        mM = nc.dram_tensor("m_red", [128, BT], F32, addr_space="Shared")
        rs_in = nc.dram_tensor("rs_in", [B, D + 1], F32)
        rs_out = nc.dram_tensor("rs_out", [128, D + 1], F32)
    else:
        S_out = nc.declare_dram_parameter("S_out", [B, D], F32, isOutput=True)
        m_out = nc.declare_dram_parameter("m_out", [128, BT], F32, isOutput=True)
        l_out = nc.declare_dram_parameter("l_out", [128, BT], F32, isOutput=True)

    nch = len(CHUNKS)
    coff = np.concatenate([[0], np.cumsum(CHUNKS)])

    with tile.TileContext(nc) as tc:
        with (
            tc.tile_pool(name="res", bufs=1) as res,
            tc.tile_pool(name="gpool", bufs=GBUFS) as gpool,
            tc.tile_pool(name="ppool", bufs=6 if RANK1_ON_PE else 3) as ppool,
            tc.tile_pool(name="small", bufs=2) as small,
            tc.tile_pool(name="ptpool", bufs=6 if RANK1_ON_PE else 3) as ptpool,
            tc.tile_pool(name="gps", bufs=4, space="PSUM") as gps,
            tc.tile_pool(name="tpsum", bufs=2, space="PSUM") as tpsum,
            tc.tile_pool(name="spsum", bufs=2, space="PSUM") as spsum,
        ):
            # ---- residents (DMA in first-use order; smalls off the sync queue) ----
            c1_t = res.tile([128, BT], F32, tag="c1")
            nc.gpsimd.dma_start(c1_t[:], c1d[:])
            id_t = res.tile([128, 128], BF16, tag="ident")
            nc.gpsimd.dma_start(id_t[:], idd[:])
            if RANK1_ON_PE:
                r1l_t = res.tile([3, B], BF16, tag="r1l")
                nc.gpsimd.dma_start(r1l_t[:], r1l[:])
                r1r_t = res.tile([3, NSH], BF16, tag="r1r")
                nc.gpsimd.dma_start(r1r_t[:], r1r[:])
            else:
                u_t = res.tile([128, BT], F32, tag="u")
                nc.gpsimd.dma_start(u_t[:], ud[:])
                beta_t = res.tile([128, NSH], F32, tag="beta")

            xT_r = res.tile([128, KD, B], F32R, tag="xT_r")
            xT_re = xT.ap().rearrange("k p b -> p k b")
            nc.sync.dma_start(xT_r[:, :, 0:128], xT_re[:, :, 0:128])

            dsT_r = res.tile([128, KD, NSH], F32R, tag="dsT_r")
            dsT_re = dsT.ap().rearrange("k p n -> p k n")
            dnat_t = res.tile([128, NK2, D], BF16, tag="dnat")
            dsn_re = dsn.ap().rearrange("k p d -> p k d")

            def load_ds_chunk(c):
                o = int(coff[c])
                w = CHUNKS[c]
                for k in range(KD):
                    nc.sync.dma_start(dsT_r[:, k, o:o + w],
                                      dsT_re[:, k, o:o + w])

            def load_beta(c):
                if RANK1_ON_PE:
                    return
                o = int(coff[c])
                w = CHUNKS[c]
                nc.sync.dma_start(beta_t[:, o:o + w], beta.ap()[:, o:o + w])

            def load_dnat(k0, k1):
                nc.sync.dma_start(dnat_t[:, k0:k1, :], dsn_re[:, k0:k1, :])

            # dsT chunks stream first (phase 0 consumes them in order; later
            # phases reuse them from SBUF), then the xT tail (needed from
            # phase 1), then dnat k-groups (needed by mm2 from phase 1 on).
            load_ds_chunk(0)
            load_ds_chunk(1)
            nc.sync.dma_start(xT_r[:, :, 128:B], xT_re[:, :, 128:B])
            for c in range(2, nch):
                load_ds_chunk(c)
            for g in range(7):
                k0 = g * 7
                load_dnat(k0, min(k0 + 7, NK2))

            m_sb = res.tile([128, BT], F32, tag="m_sb")
            l_sb = res.tile([128, BT], F32, tag="l_sb")
            if combine:
                q2_t = res.tile([128, BT], F32, tag="q2")
                nc.gpsimd.dma_start(q2_t[:], q2d[:])
                M_sb = res.tile([128, BT], F32, tag="M_sb")
                wN = res.tile([128, BT], F32, tag="wN")
                wD = res.tile([128, BT], F32, tag="wD")

            state = {}

            def emit_mm1_chunk(i, c, w):
                o = int(coff[c])
                gch, gmax = state[i][:2]
                g_ps = gps.tile([128, 512], F32, tag="gps")
                for k in range(KD):
                    nc.tensor.matmul(
                        g_ps[:, :w],
                        xT_r[:, k, i * 128:(i + 1) * 128],
                        dsT_r[:, k, o:o + w],
                        start=(k == 0), stop=(k == KD - 1 and RANK1_ON_PE is False),
                    )
                if RANK1_ON_PE:
                    nc.tensor.matmul(
                        g_ps[:, :w],
                        r1l_t[:, i * 128:(i + 1) * 128],
                        r1r_t[:, o:o + w],
                        start=False, stop=True,
                    )
                G_c = gpool.tile([128, D + 1], F32, tag="G")
                if RANK1_ON_PE:
                    nc.scalar.activation(G_c[:, :w], g_ps[:, :w],
                                         mybir.ActivationFunctionType.Copy)
                else:
                    # G = beta*u + G_xy, draining PSUM->SBUF in the same pass
                    nc.vector.scalar_tensor_tensor(
                        G_c[:, :w], beta_t[:, o:o + w], u_t[:, i:i + 1],
                        g_ps[:, :w],
                        op0=mybir.AluOpType.mult, op1=mybir.AluOpType.add,
                    )
                gch[c] = G_c
                maxq.add((i, c))

            maxq = set()

            def emit_max(i, c):
                if (i, c) not in maxq:
                    return      # already emitted (phase-1 PRE overlap)
                maxq.discard((i, c))
                gch, gmax = state[i][:2]
                w = CHUNKS[c]
                nc.vector.reduce_max(gmax[:, c:c + 1], gch[c][:, :w], axis=X,
                                     op=mybir.AluOpType.max)

            def emit_exp_head(i):
                gch, gmax = state[i]
                gm = small.tile([128, 1], F32, tag="gm")
                nc.vector.reduce_max(gm[:], gmax[:], axis=X, op=mybir.AluOpType.max)
                nc.vector.tensor_mul(m_sb[:, i:i + 1], gm[:], c1_t[:, i:i + 1])
                nb = small.tile([128, 1], F32, tag="nb")
                nc.vector.tensor_scalar_mul(nb[:], m_sb[:, i:i + 1], -1.0)
                if not combine and i == BT - 1:
                    nc.sync.dma_start(m_out.ap(), m_sb[:])
                lparts = small.tile([128, nch], F32, tag="lp")
                S_ps = spsum.tile([128, D], F32, tag="S")
                state[i] = (gch, gmax, nb, lparts, S_ps)

            ptt = {}

            def emit_exp_chunk(i, c):
                gch, gmax, nb, lparts, S_ps = state[i]
                w = CHUNKS[c]
                G_c = gch.pop(c)
                P_c = ppool.tile([128, 512], BF16, tag="P")
                nc.scalar.activation(
                    P_c[:, :w], G_c[:, :w],
                    mybir.ActivationFunctionType.Exp,
                    bias=nb[:], scale=c1_t[:, i:i + 1],
                    accum_out=lparts[:, c:c + 1],
                )
                pt = ptpool.tile([128, 4, 128], BF16, tag="pt")
                if i < TSPLIT:
                    # early tiles: PE transpose (PE is DMA-starved here and
                    # the DMA engines are busy streaming inputs)
                    for n, j0 in enumerate(range(0, w, 128)):
                        pt_ps = tpsum.tile([128, 128], BF16, tag="ptp")
                        nc.tensor.transpose(pt_ps[:], P_c[:, j0:j0 + 128],
                                            id_t[:])
                        nc.vector.tensor_copy(pt[:, n, :], pt_ps[:])
                else:
                    q = nc.scalar if (i == BT - 1 and c >= 2) else nc.sync
                    q.dma_start(pt[:, :w // 128, :], P_c[:, :w],
                                transpose=True)
                ptt[(i, c)] = pt

            def emit_mm2_chunk(i, c, w):
                gch, gmax, nb, lparts, S_ps = state[i]
                o = int(coff[c])
                pt = ptt.pop((i, c))
                for n, j0 in enumerate(range(0, w, 128)):
                    kt = (o + j0) // 128
                    nc.tensor.matmul(S_ps[:], pt[:, n, :], dnat_t[:, kt, :],
                                     start=(kt == 0), stop=(kt == NK2 - 1))

            pending_stores = []

            def emit_mm2_tail(i):
                _, _, _, lparts, S_ps = state.pop(i)
                nc.vector.reduce_sum(l_sb[:, i:i + 1], lparts[:], axis=X,
                                     op=mybir.AluOpType.add)
                if not combine and i == BT - 1:
                    nc.sync.dma_start(l_out.ap(), l_sb[:])
                S_sb = gpool.tile([128, D + 1], F32, tag="G")
                if i == BT - 1:
                    # split the final drain into pipelined column halves
                    dst = S_loc if combine else S_out
                    h = D // 2
                    nc.vector.tensor_copy(S_sb[:, :h], S_ps[:, :h])
                    nc.sync.dma_start(dst[i * 128:(i + 1) * 128, :h],
                                      S_sb[:, :h])
                    nc.vector.tensor_copy(S_sb[:, h:D], S_ps[:, h:])
                    nc.sync.dma_start(dst[i * 128:(i + 1) * 128, h:],
                                      S_sb[:, h:D])
                else:
                    nc.vector.tensor_copy(S_sb[:, :D], S_ps[:])
                    pending_stores.append((i, S_sb))

            def flush_stores():
                while pending_stores:
                    i, S_sb = pending_stores.pop(0)
                    dst = S_loc if combine else S_out
                    nc.sync.dma_start(dst[i * 128:(i + 1) * 128, :],
                                      S_sb[:, :D])

            def emit_m_collective():
                nc.sync.dma_start(mb[:], m_sb[:])
                nc.gpsimd.collective_compute(
                    "AllReduce", mybir.AluOpType.max,
                    replica_groups=[list(range(NCORES))],
                    ins=[mb.ap()], outs=[mM.ap()],
                )
                nc.sync.dma_start(M_sb[:], mM[:])
                dcol = small.tile([128, BT], F32, tag="dcol")
                nc.vector.tensor_sub(dcol[:], m_sb[:], M_sb[:])
                nc.scalar.activation(wD[:], dcol[:],
                                     mybir.ActivationFunctionType.Exp)
                nc.vector.tensor_mul(wN[:], wD[:], q2_t[:])

            def emit_rescale(i):
                F = gpool.tile([128, D + 1], F32, tag="G")
                nc.sync.dma_start(F[:, :D], S_loc[i * 128:(i + 1) * 128, :])
                nc.vector.tensor_scalar_mul(F[:, :D], F[:, :D], wN[:, i:i + 1])
                nc.vector.tensor_mul(F[:, D:D + 1], l_sb[:, i:i + 1],
                                     wD[:, i:i + 1])
                nc.sync.dma_start(rs_in[i * 128:(i + 1) * 128, :], F[:])

            def emit_final():
                nc.gpsimd.collective_compute(
                    "ReduceScatter", mybir.AluOpType.add,
                    replica_groups=[list(range(NCORES))],
                    ins=[rs_in.ap()], outs=[rs_out.ap()],
                )
                R = gpool.tile([128, D + 1], F32, tag="G")
                nc.sync.dma_start(R[:], rs_out[:])
                rec = small.tile([128, 1], F32, tag="rec")
                nc.vector.reciprocal(rec[:], R[:, D:D + 1])
                nc.vector.tensor_scalar_mul(R[:, :D], R[:, :D], rec[:])
                Xf = gpool.tile([128, D + 1], F32, tag="G")
                nc.sync.dma_start(Xf[:, :D], xq1[:])
                V = gpool.tile([128, D + 1], F32, tag="G")
                nc.vector.tensor_add(V[:, :D], R[:, :D], Xf[:, :D])
                nc.sync.dma_start(vout[:], V[:, :D])

            def alloc_tile_state(i):
                gmax = small.tile([128, nch], F32, tag="gmax")
                state[i] = ({}, gmax)

            # software-pipelined, per phase i: mm1 of tile i leads, exp of
            # tile i-1 tracks it, mm2 of tile i-1 lags by LAG chunks so the
            # phase-boundary chain (exp_head -> exp -> DMA-transpose) is
            # covered by mm1 work on the PE
            LAG = 6
            alloc_tile_state(0)
            if BT > 1:
                alloc_tile_state(1)
            for c, w in enumerate(CHUNKS):
                emit_mm1_chunk(0, c, w)
                if c < PRE:
                    emit_mm1_chunk(1, c, w)
                if c > 0:
                    emit_max(0, c - 1)
            emit_max(0, nch - 1)
            for i in range(1, BT):
                if i not in state:
                    alloc_tile_state(i)
                emit_exp_head(i - 1)
                sh = PRE if i == 1 else 0   # step-1 mm1 chunks shifted by PRE
                for k in range(sh):
                    emit_max(i, k)
                for c in range(nch):
                    if c + sh < nch:
                        emit_mm1_chunk(i, c + sh, CHUNKS[c + sh])
                    emit_exp_chunk(i - 1, c)
                    if c == 2:
                        flush_stores()
                    if c >= LAG:
                        emit_mm2_chunk(i - 1, c - LAG, CHUNKS[c - LAG])
                    if c > 0 and c - 1 + sh < nch:
                        emit_max(i, c - 1 + sh)
                emit_max(i, nch - 1)
                if i == BT - 1:
                    emit_exp_head(BT - 1)
                    emit_exp_chunk(BT - 1, 0)
                    emit_exp_chunk(BT - 1, 1)
                for c in range(nch - LAG, nch):
                    emit_mm2_chunk(i - 1, c, CHUNKS[c])
                emit_mm2_tail(i - 1)
            if BT == 1:
                emit_exp_head(0)
                emit_exp_chunk(0, 0)
                emit_exp_chunk(0, 1)
            if combine:
                emit_m_collective()
            i = BT - 1
            for c, w in enumerate(CHUNKS):
                if c + 2 < nch:
                    emit_exp_chunk(i, c + 2)
                if c == 2:
                    flush_stores()
                emit_mm2_chunk(i, c, w)
                if combine and c < BT - 1:
                    emit_rescale(c)
            emit_mm2_tail(i)
            flush_stores()
            if combine:
                emit_rescale(BT - 1)
                emit_final()

    nc.compile()
    return nc


_NC_CACHE = {}


def _get_nc(combine=True):
    if combine not in _NC_CACHE:
        _NC_CACHE[combine] = _build(combine)
    return _NC_CACHE[combine]


def _split_bf16(v):
    hi = v.astype(np.float32).astype(BF)
    lo = (v.astype(np.float64) - hi.astype(np.float64)).astype(np.float32).astype(BF)
    return hi, lo


def _prep_inputs(x_t, t, dataset, combine=True):
    x_t = np.asarray(x_t, dtype=np.float32)
    t = np.asarray(t, dtype=np.float32)
    dataset = np.asarray(dataset, dtype=np.float32)

    a = t.astype(np.float64)
    b = 1.0 - a
    c1 = np.ascontiguousarray(
        (a / (b * b)).astype(np.float32).reshape(BT, 128).T)
    u = np.ascontiguousarray(
        (-a / 2.0).astype(np.float32).reshape(BT, 128).T)

    dsp = np.full((NPAD, D), PADVAL, dtype=np.float32)
    dsp[:N] = dataset
    dsnc = ((dsp.astype(np.float64) ** 2).sum(1) - float(D)).astype(np.float32)

    if RANK1_ON_PE:
        uu = -a / 2.0
        u_hi, u_lo = _split_bf16(uu)
        r1_lhsT = np.stack([u_hi, u_lo, u_hi]).astype(BF)       # (3, B)
        v_hi, v_lo = _split_bf16(dsnc.astype(np.float64))
        r1_rhs_full = np.stack([v_hi, v_hi, v_lo]).astype(BF)   # (3, NPAD)

    xT = np.ascontiguousarray(x_t.T).reshape(KD, 128, B)
    dsT_full = np.ascontiguousarray(dsp.T)                      # (D, NPAD)
    ds_bf = dsp.astype(BF)                                      # (NPAD, D)

    q2 = np.ascontiguousarray(
        (1.0 + a / b).astype(np.float32).reshape(BT, 128).T)
    x_q1 = ((-1.0 / b)[:, None] * x_t.astype(np.float64)).astype(np.float32)

    in_maps = []
    for c in range(NCORES):
        sl = slice(c * NSH, (c + 1) * NSH)
        im = {
            "ident": np.eye(128, dtype=np.float32).astype(BF),
            "xT": xT,
            "dsT": np.ascontiguousarray(dsT_full[:, sl]).reshape(KD, 128, NSH),
            "ds_nat": np.ascontiguousarray(ds_bf[sl]).reshape(NK2, 128, D),
            "c1": c1,
        }
        if RANK1_ON_PE:
            im["r1_lhsT"] = r1_lhsT
            im["r1_rhs"] = np.ascontiguousarray(r1_rhs_full[:, sl])
        else:
            im["beta_bc"] = np.ascontiguousarray(
                np.broadcast_to(dsnc[sl], (128, NSH)))
            im["u"] = u
        if combine:
            im["q2"] = q2
            im["xq1"] = x_q1[c * 128:(c + 1) * 128, :]
        in_maps.append(im)
    return in_maps


def _combine_host(results, x_t, t):
    a = t.astype(np.float64)
    b = 1.0 - a
    m_c = np.stack([np.asarray(r["m_out"], dtype=np.float64).T.reshape(-1)
                    for r in results])                          # (8, B)
    l_c = np.stack([np.asarray(r["l_out"], dtype=np.float64).T.reshape(-1)
                    for r in results])                          # (8, B)
    S_c = np.stack([np.asarray(r["S_out"], dtype=np.float64)
                    for r in results])                          # (8, B, D)
    M = m_c.max(0)
    w = np.exp(m_c - M)                                         # (8, B)
    S = np.einsum("cb,cbd->bd", w, S_c)
    L = (w * l_c).sum(0)
    wd = S / L[:, None]
    v = (-1.0 / b)[:, None] * x_t.astype(np.float64) \
        + (1.0 + a / b)[:, None] * wd
    return v.astype(np.float32)


def run_full(x_t, t, dataset, trace=False, combine=False):
    nc = _get_nc(combine)
    in_maps = _prep_inputs(x_t, t, dataset, combine=combine)
    res = run_bass_kernel_spmd(nc, in_maps, core_ids=list(range(NCORES)),
                               trace=trace)
    if combine:
        v = np.concatenate([np.asarray(r["out"]) for r in res.results], axis=0)
    else:
        v = _combine_host(res.results, np.asarray(x_t, np.float32),
                          np.asarray(t, np.float32))
    return v, res


def kernel(x_t, t, dataset):
    v, _ = run_full(x_t, t, dataset)
    return v



# revision 5
# speedup vs baseline: 1.0481x; 1.0481x over previous
"""AnalyticGaussianVelocity Trainium2 kernel, 8 NeuronCores.

Math (reference):
    a=t, b=1-t
    logit_n = -(1/(2b^2)) * (|x|^2 - 2a x.y_n + a^2 |y_n|^2)
    v = -(1/b) x + (1 + a/b) * softmax(logit) @ dataset

Device-side per core (dataset sharded along N, padded 50000->51200, 6400/core,
free-dim chunks 12x512+256):
    G_n   = x.y_n + u*(|y_n|^2 - 512)   u=-a/2    (4 f32r matmuls -> PSUM +
                                                   a 5th 3-row split-bf16
                                                   rank-1 matmul)
    logit'_n = c1 * G_n, c1 = a/b^2
    m     = max_n logit'_n              (ACT Copy drain -> DVE chunk maxes)
    P_n   = exp(logit'_n - m)           (ACT exp, fp8-e4m3 out, f32 accum -> l)
    S^T   = sum_n P_n y_n               (fp8 DoubleRow matmuls: dataset is the
                                         stationary side split into e4m3
                                         hi+lo, P^T pairs are the moving side
                                         via u16 XBAR transposes of the fp8 P;
                                         out is S^T [d, b], 2x PE throughput
                                         vs bf16)
Host combine (flash-attention style) over the 8 core shards:
    M = max_c m_c; w = exp(m_c - M); S = sum w*S_c; L = sum w*l_c
    v = -(1/b) x + (1 + a/b) * S / L

Padding rows are the constant 2048.0 -> giant |y|^2 -> logit ~ -1e7 -> w 0.
The DoubleRow pairing groups n = 256*s + 2p + i (p = partition, i = k-group);
the u16 transpose of the fp8 P pairs adjacent n automatically, and the
dataset hi/lo tensors are pre-interleaved on the host to match.
"""

import numpy as np
import ml_dtypes

import concourse.bass as bass
from concourse import bacc
import concourse.mybir as mybir
import concourse.tile as tile
from concourse.bass_utils import run_bass_kernel_spmd

F32 = mybir.dt.float32
F32R = mybir.dt.float32r
BF16 = mybir.dt.bfloat16
FP8 = mybir.dt.float8e4
U16 = mybir.dt.uint16
BF = ml_dtypes.bfloat16
E4 = ml_dtypes.float8_e4m3
DR = mybir.MatmulPerfMode.DoubleRow

B, D, N = 1024, 512, 50000
NCORES = 8
NPAD = 51200                      # 8 * 6400, multiple of 2048
NSH = NPAD // NCORES              # 6400 per core
KD = D // 128                     # 4 contraction tiles for logits matmul
BT = B // 128                     # 8 batch tiles
CHUNKS = [512] * 12 + [256]       # free-dim chunks of NSH (>=256: f32r rate)
NSLAB = NSH // 256                # 25 DoubleRow slabs (256 n each)
DSL = D // 128                    # 4 d-slices for the S^T matmuls
PADVAL = 2048.0
X = mybir.AxisListType.X

GBUFS = 16                        # G chunks in flight (13/tile + slack)


def _build():
    nc = bacc.Bacc("TRN2", target_bir_lowering=False, debug=False,
                   num_devices=NCORES, dynamic_dma_scratch_size=512)

    xT = nc.declare_dram_parameter("xT", [KD, 128, B], F32R, isOutput=False)
    dsT = nc.declare_dram_parameter("dsT", [KD, 128, NSH], F32R, isOutput=False)
    ds_hi = nc.declare_dram_parameter("ds_hi", [128, NSLAB, 2, D], FP8,
                                      isOutput=False)
    ds_lo = nc.declare_dram_parameter("ds_lo", [128, NSLAB, 2, D], FP8,
                                      isOutput=False)
    c1d = nc.declare_dram_parameter("c1", [128, BT], F32, isOutput=False)
    r1l = nc.declare_dram_parameter("r1_lhsT", [3, B], BF16, isOutput=False)
    r1r = nc.declare_dram_parameter("r1_rhs", [3, NSH], BF16, isOutput=False)

    S_out = nc.declare_dram_parameter("S_outT", [BT, 128, DSL, 128], F32,
                                      isOutput=True)
    m_out = nc.declare_dram_parameter("m_out", [128, BT], F32, isOutput=True)
    l_out = nc.declare_dram_parameter("l_out", [128, BT], F32, isOutput=True)

    nch = len(CHUNKS)
    coff = np.concatenate([[0], np.cumsum(CHUNKS)])

    with tile.TileContext(nc) as tc:
        with (
            tc.tile_pool(name="res", bufs=1) as res,
            tc.tile_pool(name="gpool", bufs=GBUFS) as gpool,
            tc.tile_pool(name="ppool", bufs=4) as ppool,
            tc.tile_pool(name="small", bufs=2) as small,
            tc.tile_pool(name="ptpool", bufs=8) as ptpool,
            tc.tile_pool(name="gps", bufs=4, space="PSUM") as gps,
            tc.tile_pool(name="spsum", bufs=2, space="PSUM") as spsum,
        ):
            # ---- residents (DMA in first-use order; smalls off sync queue) --
            c1_t = res.tile([128, BT], F32, tag="c1")
            nc.gpsimd.dma_start(c1_t[:], c1d[:])
            r1l_t = res.tile([3, B], BF16, tag="r1l")
            nc.gpsimd.dma_start(r1l_t[:], r1l[:])
            r1r_t = res.tile([3, NSH], BF16, tag="r1r")
            nc.gpsimd.dma_start(r1r_t[:], r1r[:])

            xT_r = res.tile([128, KD, B], F32R, tag="xT_r")
            xT_re = xT.ap().rearrange("k p b -> p k b")
            nc.sync.dma_start(xT_r[:, :, 0:128], xT_re[:, :, 0:128])

            dsT_r = res.tile([128, KD, NSH], F32R, tag="dsT_r")
            dsT_re = dsT.ap().rearrange("k p n -> p k n")
            dhi_t = res.tile([128, NSLAB, 2, D], FP8, tag="dhi")
            dlo_t = res.tile([128, NSLAB, 2, D], FP8, tag="dlo")

            def load_ds_chunk(c):
                o = int(coff[c])
                w = CHUNKS[c]
                for k in range(KD):
                    nc.sync.dma_start(dsT_r[:, k, o:o + w],
                                      dsT_re[:, k, o:o + w])

            def load_ds8(s0, s1):
                nc.sync.dma_start(dhi_t[:, s0:s1], ds_hi.ap()[:, s0:s1])
                nc.sync.dma_start(dlo_t[:, s0:s1], ds_lo.ap()[:, s0:s1])

            # dsT chunks stream first (phase 0 consumes them in order), then
            # the xT tail (needed from phase 1), then ds8 slab groups (needed
            # by mm2 from phase 1 on).
            load_ds_chunk(0)
            load_ds_chunk(1)
            nc.sync.dma_start(xT_r[:, :, 128:B], xT_re[:, :, 128:B])
            for c in range(2, nch):
                load_ds_chunk(c)
            for g in range(5):
                s0 = g * 5
                load_ds8(s0, min(s0 + 5, NSLAB))

            m_sb = res.tile([128, BT], F32, tag="m_sb")
            l_sb = res.tile([128, BT], F32, tag="l_sb")

            state = {}

            def emit_mm1_chunk(i, c, w):
                o = int(coff[c])
                gch, gmax = state[i][:2]
                g_ps = gps.tile([128, 512], F32, tag="gps")
                for k in range(KD):
                    nc.tensor.matmul(
                        g_ps[:, :w],
                        xT_r[:, k, i * 128:(i + 1) * 128],
                        dsT_r[:, k, o:o + w],
                        start=(k == 0), stop=False,
                    )
                nc.tensor.matmul(
                    g_ps[:, :w],
                    r1l_t[:, i * 128:(i + 1) * 128],
                    r1r_t[:, o:o + w],
                    start=False, stop=True,
                )
                G_c = gpool.tile([128, 512], F32, tag="G")
                nc.scalar.activation(G_c[:, :w], g_ps[:, :w],
                                     mybir.ActivationFunctionType.Copy)
                gch[c] = G_c
                maxq.add((i, c))

            maxq = set()

            def emit_max(i, c):
                if (i, c) not in maxq:
                    return
                maxq.discard((i, c))
                gch, gmax = state[i][:2]
                w = CHUNKS[c]
                nc.vector.reduce_max(gmax[:, c:c + 1], gch[c][:, :w], axis=X,
                                     op=mybir.AluOpType.max)

            def emit_exp_head(i):
                gch, gmax = state[i]
                gm = small.tile([128, 1], F32, tag="gm")
                nc.vector.reduce_max(gm[:], gmax[:], axis=X,
                                     op=mybir.AluOpType.max)
                nc.vector.tensor_mul(m_sb[:, i:i + 1], gm[:], c1_t[:, i:i + 1])
                nb = small.tile([128, 1], F32, tag="nb")
                nc.vector.tensor_scalar_mul(nb[:], m_sb[:, i:i + 1], -1.0)
                if i == BT - 1:
                    nc.gpsimd.dma_start(m_out.ap(), m_sb[:])
                lparts = small.tile([128, nch], F32, tag="lp")
                S_ps = spsum.tile([128, DSL, 128], F32, tag="S")
                state[i] = (gch, gmax, nb, lparts, S_ps)

            ptt = {}

            def emit_exp_chunk(i, c):
                gch, gmax, nb, lparts, S_ps = state[i]
                w = CHUNKS[c]
                G_c = gch.pop(c)
                P_c = ppool.tile([128, 512], FP8, tag="P")
                nc.scalar.activation(
                    P_c[:, :w], G_c[:, :w],
                    mybir.ActivationFunctionType.Exp,
                    bias=nb[:], scale=c1_t[:, i:i + 1],
                    accum_out=lparts[:, c:c + 1],
                )
                nblk = w // 256
                pt = ptpool.tile([128, 2, 128], U16, tag="pt")
                q = nc.scalar if (i == BT - 1 and c >= 2) else nc.sync
                q.dma_start(pt[:, :nblk, :],
                            P_c.bitcast(U16)[:, :w // 2], transpose=True)
                ptt[(i, c)] = pt

            def emit_mm2_chunk(i, c, w):
                gch, gmax, nb, lparts, S_ps = state[i]
                pt = ptt.pop((i, c))
                # the whole [128, 4, 128] tile is one PSUM bank = one zero
                # region: a single accumulation group (start on the global
                # first matmul, stop on the global last) — per-slice groups
                # would re-zero each other's partial sums.
                for j in range(w // 256):
                    s = 2 * c + j
                    ptj = pt[:, j, :]
                    mov = ptj.bitcast(FP8).rearrange("p (b i) -> p i b", i=2)
                    for h, dst in enumerate((dhi_t, dlo_t)):
                        for e in range(DSL):
                            nc.tensor.matmul(
                                S_ps[:, e, :],
                                dst[:, s, :, e * 128:(e + 1) * 128],
                                mov,
                                start=(s == 0 and h == 0 and e == 0),
                                stop=(s == NSLAB - 1 and h == 1 and e == DSL - 1),
                                perf_mode=DR,
                                skip_group_check=True,
                            )

            pending_stores = []

            def emit_mm2_tail(i):
                _, _, _, lparts, S_ps = state.pop(i)
                nc.vector.reduce_sum(l_sb[:, i:i + 1], lparts[:], axis=X,
                                     op=mybir.AluOpType.add)
                if i == BT - 1:
                    nc.gpsimd.dma_start(l_out.ap(), l_sb[:])
                S_sb = gpool.tile([128, 512], F32, tag="G")
                S_flat = S_ps[:].rearrange("p e b -> p (e b)")
                if i == BT - 1:
                    # split the final drain into pipelined halves
                    nc.vector.tensor_copy(S_sb[:, :256], S_flat[:, :256])
                    nc.sync.dma_start(
                        S_out.ap().rearrange("i p e b -> p i (e b)")[:, i, :256],
                        S_sb[:, :256])
                    nc.vector.tensor_copy(S_sb[:, 256:], S_flat[:, 256:])
                    nc.sync.dma_start(
                        S_out.ap().rearrange("i p e b -> p i (e b)")[:, i, 256:],
                        S_sb[:, 256:])
                else:
                    nc.vector.tensor_copy(S_sb[:], S_flat)
                    pending_stores.append((i, S_sb))

            def flush_stores():
                while pending_stores:
                    i, S_sb = pending_stores.pop(0)
                    nc.sync.dma_start(
                        S_out.ap().rearrange("i p e b -> p i (e b)")[:, i, :],
                        S_sb[:])

            def alloc_tile_state(i):
                gmax = small.tile([128, nch], F32, tag="gmax")
                state[i] = ({}, gmax)

            # software-pipelined, per phase i: mm1 of tile i leads, exp of
            # tile i-1 tracks it, mm2 of tile i-1 lags by LAG chunks so the
            # phase-boundary chain (exp_head -> exp -> DMA-transpose) is
            # covered by mm1 work on the PE
            LAG = 6
            PRE = 3
            alloc_tile_state(0)
            if BT > 1:
                alloc_tile_state(1)
            for c, w in enumerate(CHUNKS):
                emit_mm1_chunk(0, c, w)
                if c < PRE:
                    emit_mm1_chunk(1, c, w)
                if c > 0:
                    emit_max(0, c - 1)
            emit_max(0, nch - 1)
            for i in range(1, BT):
                if i not in state:
                    alloc_tile_state(i)
                emit_exp_head(i - 1)
                sh = PRE if i == 1 else 0   # step-1 mm1 chunks shifted by PRE
                for k in range(sh):
                    emit_max(i, k)
                for c in range(nch):
                    if c + sh < nch:
                        emit_mm1_chunk(i, c + sh, CHUNKS[c + sh])
                    emit_exp_chunk(i - 1, c)
                    if c == 2:
                        flush_stores()
                    if c >= LAG:
                        emit_mm2_chunk(i - 1, c - LAG, CHUNKS[c - LAG])
                    if c > 0 and c - 1 + sh < nch:
                        emit_max(i, c - 1 + sh)
                emit_max(i, nch - 1)
                if i == BT - 1:
                    emit_exp_head(BT - 1)
                    emit_exp_chunk(BT - 1, 0)
                    emit_exp_chunk(BT - 1, 1)
                for c in range(nch - LAG, nch):
                    emit_mm2_chunk(i - 1, c, CHUNKS[c])
                emit_mm2_tail(i - 1)
            i = BT - 1
            for c, w in enumerate(CHUNKS):
                if c + 2 < nch:
                    emit_exp_chunk(i, c + 2)
                if c == 2:
                    flush_stores()
                emit_mm2_chunk(i, c, w)
            emit_mm2_tail(i)
            flush_stores()

    nc.compile()
    return nc


_NC_CACHE = {}


def _get_nc():
    if "nc" not in _NC_CACHE:
        _NC_CACHE["nc"] = _build()
    return _NC_CACHE["nc"]


def _split_bf16(v):
    hi = v.astype(np.float32).astype(BF)
    lo = (v.astype(np.float64) - hi.astype(np.float64)).astype(np.float32).astype(BF)
    return hi, lo


def _prep_inputs(x_t, t, dataset):
    x_t = np.asarray(x_t, dtype=np.float32)
    t = np.asarray(t, dtype=np.float32)
    dataset = np.asarray(dataset, dtype=np.float32)

    a = t.astype(np.float64)
    b = 1.0 - a
    c1 = np.ascontiguousarray(
        (a / (b * b)).astype(np.float32).reshape(BT, 128).T)

    dsp = np.full((NPAD, D), PADVAL, dtype=np.float32)
    dsp[:N] = dataset
    dsnc = ((dsp.astype(np.float64) ** 2).sum(1) - float(D)).astype(np.float32)

    uu = -a / 2.0
    u_hi, u_lo = _split_bf16(uu)
    r1_lhsT = np.stack([u_hi, u_lo, u_hi]).astype(BF)       # (3, B)
    v_hi, v_lo = _split_bf16(dsnc.astype(np.float64))
    r1_rhs_full = np.stack([v_hi, v_hi, v_lo]).astype(BF)   # (3, NPAD)

    xT = np.ascontiguousarray(x_t.T).reshape(KD, 128, B)
    dsT_full = np.ascontiguousarray(dsp.T)                  # (D, NPAD)

    # fp8 copies feed mm2 only; pad rows carry zero softmax weight but 2048
    # overflows e4m3 (max 448) into NaN, so zero them there.
    dsp8 = dsp.copy()
    dsp8[N:] = 0.0
    hi8 = dsp8.astype(E4)                                   # (NPAD, D)
    lo8 = (dsp8.astype(np.float64) - hi8.astype(np.float64)).astype(
        np.float32).astype(E4)

    in_maps = []
    for c in range(NCORES):
        sl = slice(c * NSH, (c + 1) * NSH)
        # DoubleRow pairing: n_local = 256*s + 2*p + i -> [p, s, i, d]
        hi_sh = np.ascontiguousarray(
            hi8[sl].reshape(NSLAB, 128, 2, D).transpose(1, 0, 2, 3))
        lo_sh = np.ascontiguousarray(
            lo8[sl].reshape(NSLAB, 128, 2, D).transpose(1, 0, 2, 3))
        im = {
            "xT": xT,
            "dsT": np.ascontiguousarray(dsT_full[:, sl]).reshape(KD, 128, NSH),
            "ds_hi": hi_sh,
            "ds_lo": lo_sh,
            "c1": c1,
            "r1_lhsT": r1_lhsT,
            "r1_rhs": np.ascontiguousarray(r1_rhs_full[:, sl]),
        }
        in_maps.append(im)
    return in_maps


def _combine_host(results, x_t, t):
    a = t.astype(np.float64)
    b = 1.0 - a
    m_c = np.stack([np.asarray(r["m_out"], dtype=np.float64).T.reshape(-1)
                    for r in results])                      # (8, B)
    l_c = np.stack([np.asarray(r["l_out"], dtype=np.float64).T.reshape(-1)
                    for r in results])                      # (8, B)
    # S_outT[i, p, e, bb] = S[b = i*128+bb, d = e*128+p]
    S_c = np.stack([np.asarray(r["S_outT"], dtype=np.float64)
                    .transpose(0, 3, 2, 1).reshape(B, D)
                    for r in results])                      # (8, B, D)
    M = m_c.max(0)
    w = np.exp(m_c - M)                                     # (8, B)
    S = np.einsum("cb,cbd->bd", w, S_c)
    L = (w * l_c).sum(0)
    wd = S / L[:, None]
    v = (-1.0 / b)[:, None] * x_t.astype(np.float64) \
        + (1.0 + a / b)[:, None] * wd
    return v.astype(np.float32)


def run_full(x_t, t, dataset, trace=False):
    nc = _get_nc()
    in_maps = _prep_inputs(x_t, t, dataset)
    res = run_bass_kernel_spmd(nc, in_maps, core_ids=list(range(NCORES)),
                               trace=trace)
    v = _combine_host(res.results, np.asarray(x_t, np.float32),
                      np.asarray(t, np.float32))
    return v, res


def kernel(x_t, t, dataset):
    v, _ = run_full(x_t, t, dataset)
    return v


# revision 10
# speedup vs baseline: 1.0943x; 1.0441x over previous
"""AnalyticGaussianVelocity Trainium2 kernel, 8 NeuronCores.

Math (reference):
    a=t, b=1-t
    logit_n = -(1/(2b^2)) * (|x|^2 - 2a x.y_n + a^2 |y_n|^2)
    v = -(1/b) x + (1 + a/b) * softmax(logit) @ dataset

Device-side per core (dataset sharded along N, padded 50000->51200, 6400/core,
free-dim chunks 12x512+256):
    G_n   = x.y_n + u*(|y_n|^2 - 512)   u=-a/2    (4 f32r matmuls -> PSUM +
                                                   a 5th 3-row split-bf16
                                                   rank-1 matmul)
    logit'_n = c1 * G_n, c1 = a/b^2
    m     = max_n logit'_n              (ACT Copy drain -> DVE chunk maxes)
    P_n   = exp(logit'_n - m)           (ACT exp, fp8-e4m3 out, f32 accum -> l)
    S^T   = sum_n P_n y_n               (fp8 DoubleRow matmuls: dataset is the
                                         stationary side split into e4m3
                                         hi+lo, P^T pairs are the moving side
                                         via u16 XBAR transposes of the fp8 P;
                                         out is S^T [d, b], 2x PE throughput
                                         vs bf16)
Host combine (flash-attention style) over the 8 core shards:
    M = max_c m_c; w = exp(m_c - M); S = sum w*S_c; L = sum w*l_c
    v = -(1/b) x + (1 + a/b) * S / L

Padding rows are the constant 2048.0 -> giant |y|^2 -> logit ~ -1e7 -> w 0.
The DoubleRow pairing groups n = 256*s + 2p + i (p = partition, i = k-group);
the u16 transpose of the fp8 P pairs adjacent n automatically, and the
dataset hi/lo tensors are pre-interleaved on the host to match.
"""

import numpy as np
import ml_dtypes

import concourse.bass as bass
from concourse import bacc
import concourse.mybir as mybir
import concourse.tile as tile
from concourse.bass_utils import run_bass_kernel_spmd

F32 = mybir.dt.float32
F32R = mybir.dt.float32r
BF16 = mybir.dt.bfloat16
FP8 = mybir.dt.float8e4
U16 = mybir.dt.uint16
BF = ml_dtypes.bfloat16
E4 = ml_dtypes.float8_e4m3
DR = mybir.MatmulPerfMode.DoubleRow

B, D, N = 1024, 512, 50000
NCORES = 8
NPAD = 51200                      # 8 * 6400, multiple of 2048
NSH = NPAD // NCORES              # 6400 per core
KD = D // 128                     # 4 contraction tiles for logits matmul
BT = B // 128                     # 8 batch tiles
CHUNKS = [512] * 12 + [256]       # free-dim chunks of NSH (>=256: f32r rate)
NSLAB = NSH // 256                # 25 DoubleRow slabs (256 n each)
DSL = D // 128                    # 4 d-slices for the S^T matmuls
PADVAL = 2048.0
X = mybir.AxisListType.X

GBUFS = 16                        # G chunks in flight (13/tile + slack)


def _build(combine=False):  # combine kept for test.py compat; host always combines
    nc = bacc.Bacc("TRN2", target_bir_lowering=False, debug=False,
                   num_devices=NCORES, dynamic_dma_scratch_size=512)

    xT = nc.declare_dram_parameter("xT", [KD, 128, B], F32R, isOutput=False)
    dsT = nc.declare_dram_parameter("dsT", [KD, 128, NSH], F32R, isOutput=False)
    ds_hi = nc.declare_dram_parameter("ds_hi", [128, NSLAB, 2, D], FP8,
                                      isOutput=False)
    ds_lo = nc.declare_dram_parameter("ds_lo", [128, NSLAB, 2, D], FP8,
                                      isOutput=False)
    c1d = nc.declare_dram_parameter("c1", [128, BT], F32, isOutput=False)
    r1l = nc.declare_dram_parameter("r1_lhsT", [3, B], BF16, isOutput=False)
    r1r = nc.declare_dram_parameter("r1_rhs", [3, NSH], BF16, isOutput=False)

    S_out = nc.declare_dram_parameter("S_outT", [BT, 128, DSL, 128], F32,
                                      isOutput=True)
    m_out = nc.declare_dram_parameter("m_out", [128, BT], F32, isOutput=True)
    l_out = nc.declare_dram_parameter("l_out", [128, BT], F32, isOutput=True)

    nch = len(CHUNKS)
    coff = np.concatenate([[0], np.cumsum(CHUNKS)])

    with tile.TileContext(nc) as tc:
        with (
            tc.tile_pool(name="res", bufs=1) as res,
            tc.tile_pool(name="gpool", bufs=GBUFS) as gpool,
            tc.tile_pool(name="ppool", bufs=3) as ppool,
            tc.tile_pool(name="small", bufs=2) as small,
            tc.tile_pool(name="ptpool", bufs=6) as ptpool,
            tc.tile_pool(name="gps", bufs=4, space="PSUM") as gps,
            tc.tile_pool(name="spsum", bufs=2, space="PSUM") as spsum,
        ):
            # ---- residents (DMA in first-use order; smalls off sync queue) --
            c1_t = res.tile([128, BT], F32, tag="c1")
            nc.gpsimd.dma_start(c1_t[:], c1d[:])
            r1l_t = res.tile([3, B], BF16, tag="r1l")
            nc.gpsimd.dma_start(r1l_t[:], r1l[:])
            r1r_t = res.tile([3, NSH], BF16, tag="r1r")
            nc.gpsimd.dma_start(r1r_t[:], r1r[:])

            xT_r = res.tile([128, KD, B], F32R, tag="xT_r")
            xT_re = xT.ap().rearrange("k p b -> p k b")
            # tiles 0 and 1 lead (phase 0 runs mm1 on both via PRE)
            nc.sync.dma_start(xT_r[:, :, 0:256], xT_re[:, :, 0:256])

            dsT_r = res.tile([128, KD, NSH], F32R, tag="dsT_r")
            dsT_re = dsT.ap().rearrange("k p n -> p k n")
            dhi_t = res.tile([128, NSLAB, 2, D], FP8, tag="dhi")
            dlo_t = res.tile([128, NSLAB, 2, D], FP8, tag="dlo")

            def load_ds_chunk(c):
                o = int(coff[c])
                w = CHUNKS[c]
                # one DMA per chunk covering all 4 k-tiles: HWDGE descriptor
                # generation is a serialized global resource (~625ns/DMA)
                nc.sync.dma_start(dsT_r[:, :, o:o + w], dsT_re[:, :, o:o + w])

            def load_ds8(s0, s1):
                nc.sync.dma_start(dhi_t[:, s0:s1], ds_hi.ap()[:, s0:s1])
                nc.sync.dma_start(dlo_t[:, s0:s1], ds_lo.ap()[:, s0:s1])

            # dsT chunks stream first (phase 0 consumes them in order; the
            # xT tail rides after chunk 4 — tiles 2..7 are not needed until
            # phase 1+), then ds8 slab halves (needed by mm2 from phase 1 on).
            load_ds_chunk(0)
            load_ds_chunk(1)
            load_ds_chunk(2)
            load_ds_chunk(3)
            load_ds_chunk(4)
            nc.sync.dma_start(xT_r[:, :, 256:B], xT_re[:, :, 256:B])
            for c in range(5, nch):
                load_ds_chunk(c)
            load_ds8(0, 13)
            load_ds8(13, NSLAB)

            m_sb = res.tile([128, BT], F32, tag="m_sb")
            l_sb = res.tile([128, BT], F32, tag="l_sb")

            state = {}

            def emit_mm1_chunk(i, c, w):
                o = int(coff[c])
                gch, gmax = state[i][:2]
                g_ps = gps.tile([128, 512], F32, tag="gps")
                for k in range(KD):
                    nc.tensor.matmul(
                        g_ps[:, :w],
                        xT_r[:, k, i * 128:(i + 1) * 128],
                        dsT_r[:, k, o:o + w],
                        start=(k == 0), stop=False,
                    )
                nc.tensor.matmul(
                    g_ps[:, :w],
                    r1l_t[:, i * 128:(i + 1) * 128],
                    r1r_t[:, o:o + w],
                    start=False, stop=True,
                )
                G_c = gpool.tile([128, 512], F32, tag="G")
                nc.scalar.activation(G_c[:, :w], g_ps[:, :w],
                                     mybir.ActivationFunctionType.Copy)
                gch[c] = G_c
                maxq.add((i, c))

            maxq = set()

            def emit_max(i, c):
                if (i, c) not in maxq:
                    return
                maxq.discard((i, c))
                gch, gmax = state[i][:2]
                w = CHUNKS[c]
                nc.vector.reduce_max(gmax[:, c:c + 1], gch[c][:, :w], axis=X,
                                     op=mybir.AluOpType.max)

            def emit_exp_head(i):
                gch, gmax = state[i]
                gm = small.tile([128, 1], F32, tag="gm")
                nc.vector.reduce_max(gm[:], gmax[:], axis=X,
                                     op=mybir.AluOpType.max)
                nc.vector.tensor_mul(m_sb[:, i:i + 1], gm[:], c1_t[:, i:i + 1])
                nb = small.tile([128, 1], F32, tag="nb")
                nc.vector.tensor_scalar_mul(nb[:], m_sb[:, i:i + 1], -1.0)
                if i == BT - 1:
                    nc.gpsimd.dma_start(m_out.ap(), m_sb[:])
                lparts = small.tile([128, nch], F32, tag="lp")
                S_ps = spsum.tile([128, DSL, 128], F32, tag="S")
                state[i] = (gch, gmax, nb, lparts, S_ps)

            ptt = {}
            pcur = {}

            def emit_exp_chunk(i, c):
                gch, gmax, nb, lparts, S_ps = state[i]
                w = CHUNKS[c]
                pair, half = c // 2, c % 2
                G_c = gch.pop(c)
                # two 512-chunks share one P tile so their u16-pair XBAR
                # transpose is a single DMA (HWDGE gen is globally serialized)
                if half == 0:
                    P2 = ppool.tile([128, 1024], FP8, tag="P")
                    pcur[(i, pair)] = P2
                else:
                    P2 = pcur[(i, pair)]
                nc.scalar.activation(
                    P2[:, half * 512:half * 512 + w], G_c[:, :w],
                    mybir.ActivationFunctionType.Exp,
                    bias=nb[:], scale=c1_t[:, i:i + 1],
                    accum_out=lparts[:, c:c + 1],
                )
                if half == 1 or c == nch - 1:
                    wp = half * 512 + w
                    pt = ptpool.tile([128, 4, 128], U16, tag="pt")
                    nc.sync.dma_start(pt[:, :wp // 256, :],
                                      P2.bitcast(U16)[:, :wp // 2],
                                      transpose=True)
                    ptt[(i, pair)] = pt
                    pcur.pop((i, pair))

            def emit_mm2_chunk(i, c, w):
                gch, gmax, nb, lparts, S_ps = state[i]
                pt = ptt[(i, c // 2)]
                # the whole [128, 4, 128] tile is one PSUM bank = one zero
                # region: a single accumulation group (start on the global
                # first matmul, stop on the global last) — per-slice groups
                # would re-zero each other's partial sums.
                for j in range(w // 256):
                    s = 2 * c + j
                    blk = 2 * (c % 2) + j
                    ptj = pt[:, blk, :]
                    mov = ptj.bitcast(FP8).rearrange("p (b i) -> p i b", i=2)
                    for h, dst in enumerate((dhi_t, dlo_t)):
                        for e in range(DSL):
                            nc.tensor.matmul(
                                S_ps[:, e, :],
                                dst[:, s, :, e * 128:(e + 1) * 128],
                                mov,
                                start=(s == 0 and h == 0 and e == 0),
                                stop=(s == NSLAB - 1 and h == 1 and e == DSL - 1),
                                perf_mode=DR,
                                skip_group_check=True,
                            )
                if c % 2 == 1 or c == nch - 1:
                    ptt.pop((i, c // 2))

            pending_stores = []

            def emit_mm2_tail(i):
                _, _, _, lparts, S_ps = state.pop(i)
                nc.vector.reduce_sum(l_sb[:, i:i + 1], lparts[:], axis=X,
                                     op=mybir.AluOpType.add)
                if i == BT - 1:
                    nc.gpsimd.dma_start(l_out.ap(), l_sb[:])
                S_sb = gpool.tile([128, 512], F32, tag="G")
                S_flat = S_ps[:].rearrange("p e b -> p (e b)")
                if i == BT - 1:
                    # split the final drain into pipelined halves
                    nc.vector.tensor_copy(S_sb[:, :256], S_flat[:, :256])
                    nc.sync.dma_start(
                        S_out.ap().rearrange("i p e b -> p i (e b)")[:, i, :256],
                        S_sb[:, :256])
                    nc.vector.tensor_copy(S_sb[:, 256:], S_flat[:, 256:])
                    nc.sync.dma_start(
                        S_out.ap().rearrange("i p e b -> p i (e b)")[:, i, 256:],
                        S_sb[:, 256:])
                else:
                    nc.vector.tensor_copy(S_sb[:], S_flat)
                    pending_stores.append((i, S_sb))

            def flush_stores():
                while pending_stores:
                    i, S_sb = pending_stores.pop(0)
                    nc.sync.dma_start(
                        S_out.ap().rearrange("i p e b -> p i (e b)")[:, i, :],
                        S_sb[:])

            def alloc_tile_state(i):
                gmax = small.tile([128, nch], F32, tag="gmax")
                state[i] = ({}, gmax)

            # software-pipelined, per phase i: mm1 of tile i leads, exp of
            # tile i-1 tracks it, mm2 of tile i-1 lags by LAG chunks so the
            # phase-boundary chain (exp_head -> exp -> DMA-transpose) is
            # covered by mm1 work on the PE
            LAG = 6
            PRE = 3
            alloc_tile_state(0)
            if BT > 1:
                alloc_tile_state(1)
            for c, w in enumerate(CHUNKS):
                emit_mm1_chunk(0, c, w)
                if c < PRE:
                    emit_mm1_chunk(1, c, w)
                if c > 0:
                    emit_max(0, c - 1)
            emit_max(0, nch - 1)
            for i in range(1, BT):
                if i not in state:
                    alloc_tile_state(i)
                emit_exp_head(i - 1)
                sh = PRE if i == 1 else 0   # step-1 mm1 chunks shifted by PRE
                for k in range(sh):
                    emit_max(i, k)
                for c in range(nch):
                    if c + sh < nch:
                        emit_mm1_chunk(i, c + sh, CHUNKS[c + sh])
                    emit_exp_chunk(i - 1, c)
                    if c == 2:
                        flush_stores()
                    if c >= LAG:
                        emit_mm2_chunk(i - 1, c - LAG, CHUNKS[c - LAG])
                    if c > 0 and c - 1 + sh < nch:
                        emit_max(i, c - 1 + sh)
                emit_max(i, nch - 1)
                if i == BT - 1:
                    emit_exp_head(BT - 1)
                    emit_exp_chunk(BT - 1, 0)
                    emit_exp_chunk(BT - 1, 1)
                for c in range(nch - LAG, nch):
                    emit_mm2_chunk(i - 1, c, CHUNKS[c])
                emit_mm2_tail(i - 1)
            i = BT - 1
            for c, w in enumerate(CHUNKS):
                if c + 2 < nch:
                    emit_exp_chunk(i, c + 2)
                if c == 2:
                    flush_stores()
                emit_mm2_chunk(i, c, w)
            emit_mm2_tail(i)
            flush_stores()

    nc.compile()
    return nc


_NC_CACHE = {}


def _get_nc():
    if "nc" not in _NC_CACHE:
        _NC_CACHE["nc"] = _build()
    return _NC_CACHE["nc"]


def _split_bf16(v):
    hi = v.astype(np.float32).astype(BF)
    lo = (v.astype(np.float64) - hi.astype(np.float64)).astype(np.float32).astype(BF)
    return hi, lo


def _prep_inputs(x_t, t, dataset):
    x_t = np.asarray(x_t, dtype=np.float32)
    t = np.asarray(t, dtype=np.float32)
    dataset = np.asarray(dataset, dtype=np.float32)

    a = t.astype(np.float64)
    b = 1.0 - a
    c1 = np.ascontiguousarray(
        (a / (b * b)).astype(np.float32).reshape(BT, 128).T)

    dsp = np.full((NPAD, D), PADVAL, dtype=np.float32)
    dsp[:N] = dataset
    dsnc = ((dsp.astype(np.float64) ** 2).sum(1) - float(D)).astype(np.float32)

    uu = -a / 2.0
    u_hi, u_lo = _split_bf16(uu)
    r1_lhsT = np.stack([u_hi, u_lo, u_hi]).astype(BF)       # (3, B)
    v_hi, v_lo = _split_bf16(dsnc.astype(np.float64))
    r1_rhs_full = np.stack([v_hi, v_hi, v_lo]).astype(BF)   # (3, NPAD)

    xT = np.ascontiguousarray(x_t.T).reshape(KD, 128, B)
    dsT_full = np.ascontiguousarray(dsp.T)                  # (D, NPAD)

    # fp8 copies feed mm2 only; pad rows carry zero softmax weight but 2048
    # overflows e4m3 (max 448) into NaN, so zero them there.
    dsp8 = dsp.copy()
    dsp8[N:] = 0.0
    hi8 = dsp8.astype(E4)                                   # (NPAD, D)
    lo8 = (dsp8.astype(np.float64) - hi8.astype(np.float64)).astype(
        np.float32).astype(E4)

    in_maps = []
    for c in range(NCORES):
        sl = slice(c * NSH, (c + 1) * NSH)
        # DoubleRow pairing: n_local = 256*s + 2*p + i -> [p, s, i, d]
        hi_sh = np.ascontiguousarray(
            hi8[sl].reshape(NSLAB, 128, 2, D).transpose(1, 0, 2, 3))
        lo_sh = np.ascontiguousarray(
            lo8[sl].reshape(NSLAB, 128, 2, D).transpose(1, 0, 2, 3))
        im = {
            "xT": xT,
            "dsT": np.ascontiguousarray(dsT_full[:, sl]).reshape(KD, 128, NSH),
            "ds_hi": hi_sh,
            "ds_lo": lo_sh,
            "c1": c1,
            "r1_lhsT": r1_lhsT,
            "r1_rhs": np.ascontiguousarray(r1_rhs_full[:, sl]),
        }
        in_maps.append(im)
    return in_maps


def _combine_host(results, x_t, t):
    a = t.astype(np.float64)
    b = 1.0 - a
    m_c = np.stack([np.asarray(r["m_out"], dtype=np.float64).T.reshape(-1)
                    for r in results])                      # (8, B)
    l_c = np.stack([np.asarray(r["l_out"], dtype=np.float64).T.reshape(-1)
                    for r in results])                      # (8, B)
    # S_outT[i, p, e, bb] = S[b = i*128+bb, d = e*128+p]
    S_c = np.stack([np.asarray(r["S_outT"], dtype=np.float64)
                    .transpose(0, 3, 2, 1).reshape(B, D)
                    for r in results])                      # (8, B, D)
    M = m_c.max(0)
    w = np.exp(m_c - M)                                     # (8, B)
    S = np.einsum("cb,cbd->bd", w, S_c)
    L = (w * l_c).sum(0)
    wd = S / L[:, None]
    v = (-1.0 / b)[:, None] * x_t.astype(np.float64) \
        + (1.0 + a / b)[:, None] * wd
    return v.astype(np.float32)


def run_full(x_t, t, dataset, trace=False):
    nc = _get_nc()
    in_maps = _prep_inputs(x_t, t, dataset)
    res = run_bass_kernel_spmd(nc, in_maps, core_ids=list(range(NCORES)),
                               trace=trace)
    v = _combine_host(res.results, np.asarray(x_t, np.float32),
                      np.asarray(t, np.float32))
    return v, res


def kernel(x_t, t, dataset):
    v, _ = run_full(x_t, t, dataset)
    return v
